# revision 9
# baseline (speedup 1.0000x reference)
"""LSTM cell (4096x1024, H=1024) as a Bass/Tile kernel on 8 TRN2 NeuronCores.

Sharding: 2D grid — 4 batch-quarters x 2 H-halves. Core c = 2*i + j gets
batch rows [i*1024,(i+1)*1024) and gate-output columns [j*512,(j+1)*512).

Transposed bf16 formulation: the host pre-transposes activations so every
GEMM operand lands in SBUF in its natural matmul layout —
  gates_T[n, m] = sum_k W[k, n] * xT[k, m] + sum_k U[k, n] * hT[k, m]
with W/U k-tiles as the stationary operand (K on partitions, native [E, HL]
layout) and xT/hT as the moving operand. No on-chip transposes at all.
GEMM inputs are bf16 (1 PE cycle/row vs 4 for fp32); accumulation stays fp32
in PSUM. The per-gate bias rides the activation op as a per-partition bias AP
(gate-output dim is the partition dim here), so no K=1 seed matmuls.

Gate phases stream in weight-arrival order g = cbar, i, f, o; the gating
elementwise work attaches to each phase (i: cbar*=i; f: c = f*c_prev + cbar,
tanh(c); o: h = o*tanh(c)) so only the o-phase epilogue trails the last
matmul. h/c are produced transposed [HL, BL]; the host transposes back.
"""

import numpy as np
import ml_dtypes
from contextlib import ExitStack

import bass_rust
import concourse.bass as bass
import concourse.mybir as mybir
import concourse.tile as tile
from concourse.vector_clock import ScopedClock
from concourse.bass_utils import run_bass_kernel_spmd

f32 = mybir.dt.float32
bf16 = mybir.dt.bfloat16
AFT = mybir.ActivationFunctionType
P = 128

B, E, H = 4096, 1024, 1024
BB, BH = 4, 2              # batch quarters x H halves
BL = B // BB               # 1024 rows per core
HL = H // BH               # 512 gate cols per core
NG = 4                     # gate order: cbar, i, f, o
MBL = 512                  # moving-dim chunk per matmul (one fp32 PSUM bank)


class PatchedTC(tile.TileContext):
    # This neuronxcc's core_v3 CTRL (Drain/NoOp) struct carries no sync-wait
    # slots, so the Tile tail-drain's waits must ride on EVSEM instructions.
    def _drain_and_barrier(self, tick_clock, wait_clock):
        tmp = mybir.InstNoOp(name=f"I-{self.nc.next_id()}",
                             engine=mybir.EngineType.SP)
        wait_clock.add_sem_waits(tmp, ScopedClock({None: tick_clock.global_clock}))
        sync = tmp.sync_info
        if sync is not None:
            for w in sync.on_wait:
                sem = bass_rust.SemaphoreHandle(w.ant_name, w.id)
                self.nc.sync.wait_ge(sem, w.wait_value)
        self.nc.sync.drain()
        self.nc.all_engine_barrier()
        popped = self.nc._tile_sem_poison_stack.pop()
        assert popped is self._sem_poison
        self.nc.clear_and_free_semaphores(list(self.sems.allocated().values()))
        self.nc.all_engine_barrier()


_SPLIT_SEQ = [0]


def split_multiwaits(nc, default_max=1, opcode_max=None):
    """This walrus build accepts at most one sync wait per instruction (zero
    for CTRL-struct ops like Drain/NoOp). Move excess waits onto dedicated
    EventSemaphore instructions inserted just before, on the same engine —
    semantically identical on an in-order engine queue."""
    opcode_max = opcode_max or {"Drain": 0, "NoOp": 0}
    for fn in nc.m.functions:
        for blk in fn.blocks:
            cur = blk.instructions
            out, changed = [], False
            for ins in cur:
                si = ins.sync_info
                waits = list(si.on_wait) if si is not None and si.on_wait else []
                cap = opcode_max.get(ins.opcode, default_max)
                if len(waits) > cap:
                    keep = waits[len(waits) - cap:] if cap else []
                    spill = waits[:len(waits) - cap]
                    for w in spill:
                        _SPLIT_SEQ[0] += 1
                        ev = mybir.InstEventSemaphore(
                            name=f"I-evw{_SPLIT_SEQ[0]}", engine=ins.engine)
                        ev.sync_info = bass_rust.SyncInfo(
                            on_wait=[w], on_update=[])
                        out.append(ev)
                    ins.sync_info = bass_rust.SyncInfo(
                        on_wait=keep, on_update=list(si.on_update))
                    changed = True
                out.append(ins)
            if changed:
                blk.instructions = out
    return nc


def build_nc(bl=BL, e=E, h=H, hl=HL, split=True, repeat=1, mode="full"):
    ke, kh = e // P, h // P
    nn = hl // P               # gate-col tiles per gate (4)
    nm = bl // MBL             # moving halves per chain (2)
    nc = bass.Bass(target_bir_lowering=False)
    xt_d = nc.declare_dram_parameter("xt", [e, bl], bf16, isOutput=False)
    ht_d = nc.declare_dram_parameter("ht", [h, bl], bf16, isOutput=False)
    ct_d = nc.declare_dram_parameter("ct", [hl, bl], f32, isOutput=False)
    w_d = nc.declare_dram_parameter("w", [NG, e, hl], bf16, isOutput=False)
    u_d = nc.declare_dram_parameter("u", [NG, h, hl], bf16, isOutput=False)
    b_d = nc.declare_dram_parameter("b", [P, NG * nn], f32, isOutput=False)
    hout_d = nc.declare_dram_parameter("h_out", [hl, bl], f32, isOutput=True)
    cout_d = nc.declare_dram_parameter("c_out", [hl, bl], f32, isOutput=True)

    with PatchedTC(nc) as tc:
        with ExitStack() as ctx:
            # xt/ht double-buffered so repeat r+1's activation loads stream
            # during repeat r's compute (their last readers are r's final
            # matmul chains; bufs=1 would serialize DMA behind the whole PE
            # phase every iteration).
            actp = ctx.enter_context(tc.tile_pool(name="actp", bufs=2))
            persist = ctx.enter_context(tc.tile_pool(name="persist", bufs=1))
            wu = ctx.enter_context(tc.tile_pool(name="wu", bufs=1))
            gatep = ctx.enter_context(tc.tile_pool(name="gatep", bufs=1))
            work = ctx.enter_context(tc.tile_pool(name="work", bufs=2))
            outp = ctx.enter_context(tc.tile_pool(name="outp", bufs=2))
            gpsum = ctx.enter_context(
                tc.tile_pool(name="gpsum", bufs=4, space="PSUM"))

            def emit_body():
                xt = actp.tile([P, ke, bl], bf16, tag="xt")
                ht = actp.tile([P, kh, bl], bf16, tag="ht")
                ct = persist.tile([P, nn, bl], f32)
                bsb = persist.tile([P, NG * nn], f32)
                w_sb = [None] * NG
                u_sb = [None] * NG

                def load_w(g):
                    t = wu.tile([P, ke, hl], bf16, tag=f"w{g}")
                    nc.sync.dma_start(
                        t[:], w_d[g].rearrange("(k p) c -> p k c", p=P))
                    w_sb[g] = t

                def load_u(g):
                    t = wu.tile([P, kh, hl], bf16, tag=f"u{g}")
                    nc.sync.dma_start(
                        t[:], u_d[g].rearrange("(k p) c -> p k c", p=P))
                    u_sb[g] = t

                # DMA issue order ~ first-use order. xt and w0 stream as
                # interleaved k-granular pieces so the first chain's k=0
                # matmul can start after ~2 small transfers instead of
                # waiting out 3 MiB; ht/u0 arrive while x-products stream.
                w_sb[0] = wu.tile([P, ke, hl], bf16, tag="w0", name="w0")
                for k in range(ke):
                    nc.sync.dma_start(xt[:, k, :], xt_d[k * P:(k + 1) * P, :])
                    nc.sync.dma_start(w_sb[0][:, k, :],
                                      w_d[0, k * P:(k + 1) * P, :])
                nc.sync.dma_start(bsb[:], b_d[:, :])
                u_sb[0] = wu.tile([P, kh, hl], bf16, tag="u0", name="u0")
                for k in range(kh):
                    nc.sync.dma_start(ht[:, k, :], ht_d[k * P:(k + 1) * P, :])
                    nc.sync.dma_start(u_sb[0][:, k, :],
                                      u_d[0, k * P:(k + 1) * P, :])
                for g in range(1, NG):
                    load_w(g)
                    load_u(g)
                nc.sync.dma_start(ct[:], ct_d.rearrange("(n p) c -> p n c", p=P))

                if mode == "dma":
                    return
                cb = [gatep.tile([P, bl], bf16, tag=f"cb{n}", name=f"cb{n}")
                      for n in range(nn)]
                tnc = [gatep.tile([P, bl], bf16, tag=f"tc{n}", name=f"tc{n}")
                       for n in range(nn)]

                for g in range(NG):
                    for n in range(nn):
                        ncol = slice(n * P, (n + 1) * P)
                        ps = gpsum.tile([P, bl], f32, tag="gp")
                        for m in range(nm):
                            mo = slice(m * MBL, (m + 1) * MBL)
                            for k in range(ke):
                                nc.tensor.matmul(
                                    ps[:, mo], w_sb[g][:, k, ncol],
                                    xt[:, k, mo], start=(k == 0), stop=False)
                            for k in range(kh):
                                nc.tensor.matmul(
                                    ps[:, mo], u_sb[g][:, k, ncol],
                                    ht[:, k, mo], start=False, stop=(k == kh - 1))
                        if mode == "mm":
                            continue
                        # Output DMAs ride the ACT engine's HWDGE queue: they
                        # wait on mid-kernel DVE results, and on the (FIFO)
                        # sync queue they would head-of-line-block the next
                        # repeat's input prefetch.
                        bias = bsb[:, g * nn + n:g * nn + n + 1]
                        if g == 0:
                            nc.scalar.activation(cb[n][:], ps[:], AFT.Tanh,
                                                 bias=bias)
                        elif g == 1:
                            it = work.tile([P, bl], bf16, tag="it")
                            nc.scalar.activation(it[:], ps[:], AFT.Sigmoid,
                                                 bias=bias)
                            nc.vector.tensor_mul(cb[n][:], it[:], cb[n][:])
                        elif g == 2:
                            ft = work.tile([P, bl], bf16, tag="ft")
                            nc.scalar.activation(ft[:], ps[:], AFT.Sigmoid,
                                                 bias=bias)
                            cblk = outp.tile([P, bl], f32, tag="co")
                            nc.vector.tensor_mul(cblk[:], ft[:], ct[:, n, :])
                            nc.vector.tensor_add(cblk[:], cblk[:], cb[n][:])
                            nc.scalar.dma_start(cout_d[n * P:(n + 1) * P, :],
                                                cblk[:])
                            nc.scalar.activation(tnc[n][:], cblk[:], AFT.Tanh)
                        else:
                            ot = work.tile([P, bl], bf16, tag="ot")
                            nc.scalar.activation(ot[:], ps[:], AFT.Sigmoid,
                                                 bias=bias)
                            hblk = outp.tile([P, bl], f32, tag="ho")
                            nc.vector.tensor_mul(hblk[:], ot[:], tnc[n][:])
                            nc.scalar.dma_start(hout_d[n * P:(n + 1) * P, :],
                                                hblk[:])

            for _ in range(repeat):
                emit_body()
    return split_multiwaits(nc) if split else nc


_NC_CACHE = {}


def _get_nc(key=(BL, E, H, HL)):
    if key not in _NC_CACHE:
        _NC_CACHE[key] = build_nc(*key)
    return _NC_CACHE[key]


def make_in_maps(x, h_prev, c_prev, W, U, b):
    """W/U: [NG, E|H, H] stacked gate-major (cbar, i, f, o); b: [NG, H]."""
    bf = ml_dtypes.bfloat16
    nn = HL // P
    in_maps = []
    for core in range(BB * BH):
        i, j = divmod(core, BH)
        rs = slice(i * BL, (i + 1) * BL)
        cs = slice(j * HL, (j + 1) * HL)
        # bias as [128, NG*nn]: column t = g*nn + n holds the 128 bias values
        # for gate g, gate-col tile n — per-partition scalars for the ACT op.
        bcol = np.ascontiguousarray(b[:, cs]).reshape(NG * nn, P).T
        in_maps.append({
            "xt": np.ascontiguousarray(x[rs].T).astype(bf),
            "ht": np.ascontiguousarray(h_prev[rs].T).astype(bf),
            "ct": np.ascontiguousarray(c_prev[rs, cs].T),
            "w": np.ascontiguousarray(W[:, :, cs]).astype(bf),
            "u": np.ascontiguousarray(U[:, :, cs]).astype(bf),
            "b": np.ascontiguousarray(bcol),
        })
    return in_maps


def kernel(**inputs):
    x = np.asarray(inputs["x"], np.float32)
    hm = np.asarray(inputs["hidden_memory_tm1"], np.float32)
    h_prev, c_prev = hm[0], hm[1]
    W = np.stack([np.asarray(inputs[k], np.float32)
                  for k in ("Wc", "Wi", "Wf", "Wog")])
    U = np.stack([np.asarray(inputs[k], np.float32)
                  for k in ("Uc", "Ui", "Uf", "Uog")])
    b = np.stack([np.asarray(inputs[k], np.float32)
                  for k in ("bc", "bi", "bf", "bog")])

    nc = _get_nc()
    res = run_bass_kernel_spmd(nc, make_in_maps(x, h_prev, c_prev, W, U, b),
                               list(range(BB * BH)))
    h = np.empty((B, H), np.float32)
    c = np.empty((B, H), np.float32)
    for core in range(BB * BH):
        i, j = divmod(core, BH)
        rs = slice(i * BL, (i + 1) * BL)
        cs = slice(j * HL, (j + 1) * HL)
        h[rs, cs] = res.results[core]["h_out"].T
        c[rs, cs] = res.results[core]["c_out"].T
    return np.stack([h, c])


# revision 12
# speedup vs baseline: 1.0717x; 1.0717x over previous
"""LSTM cell (4096x1024, H=1024) as a Bass/Tile kernel on 8 TRN2 NeuronCores.

Sharding: 2D grid — 4 batch-quarters x 2 H-halves. Core c = 2*i + j gets
batch rows [i*1024,(i+1)*1024) and gate-output columns [j*512,(j+1)*512).

Transposed bf16 formulation: the host pre-transposes activations so every
GEMM operand lands in SBUF in its natural matmul layout —
  gates_T[n, m] = sum_k W[k, n] * xT[k, m] + sum_k U[k, n] * hT[k, m]
with W/U k-tiles as the stationary operand (K on partitions, native [E, HL]
layout) and xT/hT as the moving operand. No on-chip transposes at all.
GEMM inputs are bf16 (1 PE cycle/row vs 4 for fp32); accumulation stays fp32
in PSUM. The per-gate bias rides the activation op as a per-partition bias AP
(gate-output dim is the partition dim here), so no K=1 seed matmuls.

Gate phases stream in weight-arrival order g = cbar, i, f, o; the gating
elementwise work attaches to each phase (i: cbar*=i; f: c = f*c_prev + cbar,
tanh(c); o: h = o*tanh(c)) so only the o-phase epilogue trails the last
matmul. h/c are produced transposed [HL, BL]; the host transposes back.
"""

import numpy as np
import ml_dtypes
from contextlib import ExitStack

import bass_rust
import concourse.bass as bass
import concourse.mybir as mybir
import concourse.tile as tile
from concourse.vector_clock import ScopedClock
from concourse.bass_utils import run_bass_kernel_spmd

f32 = mybir.dt.float32
bf16 = mybir.dt.bfloat16
AFT = mybir.ActivationFunctionType
P = 128

B, E, H = 4096, 1024, 1024
BB, BH = 4, 2              # batch quarters x H halves
BL = B // BB               # 1024 rows per core
HL = H // BH               # 512 gate cols per core
NG = 4                     # gate order: cbar, i, f, o
MBL = 512                  # moving-dim chunk per matmul (one fp32 PSUM bank)


class PatchedTC(tile.TileContext):
    # This neuronxcc's core_v3 CTRL (Drain/NoOp) struct carries no sync-wait
    # slots, so the Tile tail-drain's waits must ride on EVSEM instructions.
    def _drain_and_barrier(self, tick_clock, wait_clock):
        tmp = mybir.InstNoOp(name=f"I-{self.nc.next_id()}",
                             engine=mybir.EngineType.SP)
        wait_clock.add_sem_waits(tmp, ScopedClock({None: tick_clock.global_clock}))
        sync = tmp.sync_info
        if sync is not None:
            for w in sync.on_wait:
                sem = bass_rust.SemaphoreHandle(w.ant_name, w.id)
                self.nc.sync.wait_ge(sem, w.wait_value)
        self.nc.sync.drain()
        self.nc.all_engine_barrier()
        popped = self.nc._tile_sem_poison_stack.pop()
        assert popped is self._sem_poison
        self.nc.clear_and_free_semaphores(list(self.sems.allocated().values()))
        self.nc.all_engine_barrier()


_SPLIT_SEQ = [0]


def split_multiwaits(nc, default_max=1, opcode_max=None):
    """This walrus build accepts at most one sync wait per instruction (zero
    for CTRL-struct ops like Drain/NoOp). Move excess waits onto dedicated
    EventSemaphore instructions inserted just before, on the same engine —
    semantically identical on an in-order engine queue."""
    opcode_max = opcode_max or {"Drain": 0, "NoOp": 0}
    for fn in nc.m.functions:
        for blk in fn.blocks:
            cur = blk.instructions
            out, changed = [], False
            for ins in cur:
                si = ins.sync_info
                waits = list(si.on_wait) if si is not None and si.on_wait else []
                cap = opcode_max.get(ins.opcode, default_max)
                if len(waits) > cap:
                    keep = waits[len(waits) - cap:] if cap else []
                    spill = waits[:len(waits) - cap]
                    for w in spill:
                        _SPLIT_SEQ[0] += 1
                        ev = mybir.InstEventSemaphore(
                            name=f"I-evw{_SPLIT_SEQ[0]}", engine=ins.engine)
                        ev.sync_info = bass_rust.SyncInfo(
                            on_wait=[w], on_update=[])
                        out.append(ev)
                    ins.sync_info = bass_rust.SyncInfo(
                        on_wait=keep, on_update=list(si.on_update))
                    changed = True
                out.append(ins)
            if changed:
                blk.instructions = out
    return nc


def build_nc(bl=BL, e=E, h=H, hl=HL, split=True, repeat=1, mode="full",
             chain="mk"):
    # chain="km" (consecutive matmuls share the stationary tile via two
    # interleaved accumulation groups) measured ~7us SLOWER on HW than the
    # plain m-outer order — weight loads hide fully under the 512-row
    # moving stream, so keep "mk".
    ke, kh = e // P, h // P
    nn = hl // P               # gate-col tiles per gate (4)
    nm = bl // MBL             # moving halves per chain (2)
    nc = bass.Bass(target_bir_lowering=False)
    xt_d = nc.declare_dram_parameter("xt", [e, bl], bf16, isOutput=False)
    ht_d = nc.declare_dram_parameter("ht", [h, bl], bf16, isOutput=False)
    ct_d = nc.declare_dram_parameter("ct", [hl, bl], f32, isOutput=False)
    w_d = nc.declare_dram_parameter("w", [NG, e, hl], bf16, isOutput=False)
    u_d = nc.declare_dram_parameter("u", [NG, h, hl], bf16, isOutput=False)
    b_d = nc.declare_dram_parameter("b", [P, NG * nn], f32, isOutput=False)
    hout_d = nc.declare_dram_parameter("h_out", [hl, bl], f32, isOutput=True)
    cout_d = nc.declare_dram_parameter("c_out", [hl, bl], f32, isOutput=True)

    with PatchedTC(nc) as tc:
        with ExitStack() as ctx:
            # xt/ht double-buffered so repeat r+1's activation loads stream
            # during repeat r's compute (their last readers are r's final
            # matmul chains; bufs=1 would serialize DMA behind the whole PE
            # phase every iteration).
            actp = ctx.enter_context(tc.tile_pool(name="actp", bufs=2))
            persist = ctx.enter_context(tc.tile_pool(name="persist", bufs=1))
            wu = ctx.enter_context(tc.tile_pool(name="wu", bufs=1))
            gatep = ctx.enter_context(tc.tile_pool(name="gatep", bufs=1))
            work = ctx.enter_context(tc.tile_pool(name="work", bufs=2))
            outp = ctx.enter_context(tc.tile_pool(name="outp", bufs=2))
            gpsum = ctx.enter_context(
                tc.tile_pool(name="gpsum", bufs=4, space="PSUM"))

            def emit_body():
                xt = actp.tile([P, ke, bl], bf16, tag="xt")
                ht = actp.tile([P, kh, bl], bf16, tag="ht")
                ct = persist.tile([P, nn, bl], f32)
                bsb = persist.tile([P, NG * nn], f32)
                w_sb = [None] * NG
                u_sb = [None] * NG

                def load_w(g):
                    t = wu.tile([P, ke, hl], bf16, tag=f"w{g}")
                    nc.sync.dma_start(
                        t[:], w_d[g].rearrange("(k p) c -> p k c", p=P))
                    w_sb[g] = t

                def load_u(g):
                    t = wu.tile([P, kh, hl], bf16, tag=f"u{g}")
                    nc.sync.dma_start(
                        t[:], u_d[g].rearrange("(k p) c -> p k c", p=P))
                    u_sb[g] = t

                # DMA issue order ~ first-use order. xt and w0 stream as
                # interleaved k-granular pieces so the first chain's k=0
                # matmul can start after ~2 small transfers instead of
                # waiting out 3 MiB; ht/u0 arrive while x-products stream.
                w_sb[0] = wu.tile([P, ke, hl], bf16, tag="w0", name="w0")
                for k in range(ke):
                    nc.sync.dma_start(xt[:, k, :], xt_d[k * P:(k + 1) * P, :])
                    nc.sync.dma_start(w_sb[0][:, k, :],
                                      w_d[0, k * P:(k + 1) * P, :])
                nc.sync.dma_start(bsb[:], b_d[:, :])
                u_sb[0] = wu.tile([P, kh, hl], bf16, tag="u0", name="u0")
                for k in range(kh):
                    nc.sync.dma_start(ht[:, k, :], ht_d[k * P:(k + 1) * P, :])
                    nc.sync.dma_start(u_sb[0][:, k, :],
                                      u_d[0, k * P:(k + 1) * P, :])
                for g in range(1, NG):
                    load_w(g)
                    load_u(g)
                nc.sync.dma_start(ct[:], ct_d.rearrange("(n p) c -> p n c", p=P))

                if mode == "dma":
                    return
                cb = [gatep.tile([P, bl], bf16, tag=f"cb{n}", name=f"cb{n}")
                      for n in range(nn)]
                tnc = [gatep.tile([P, bl], bf16, tag=f"tc{n}", name=f"tc{n}")
                       for n in range(nn)]

                for g in range(NG):
                    for n in range(nn):
                        ncol = slice(n * P, (n + 1) * P)
                        ps = gpsum.tile([P, bl], f32, tag="gp")
                        if chain == "km":
                            # k-outer, m-inner: consecutive matmuls share the
                            # same stationary W/U tile, so the PE reloads
                            # weights every 2nd instruction instead of every
                            # instruction (two interleaved accumulation
                            # groups, one per PSUM bank).
                            for k in range(ke):
                                for m in range(nm):
                                    mo = slice(m * MBL, (m + 1) * MBL)
                                    nc.tensor.matmul(
                                        ps[:, mo], w_sb[g][:, k, ncol],
                                        xt[:, k, mo], start=(k == 0),
                                        stop=False)
                            for k in range(kh):
                                for m in range(nm):
                                    mo = slice(m * MBL, (m + 1) * MBL)
                                    nc.tensor.matmul(
                                        ps[:, mo], u_sb[g][:, k, ncol],
                                        ht[:, k, mo], start=False,
                                        stop=(k == kh - 1))
                        else:
                            for m in range(nm):
                                mo = slice(m * MBL, (m + 1) * MBL)
                                for k in range(ke):
                                    nc.tensor.matmul(
                                        ps[:, mo], w_sb[g][:, k, ncol],
                                        xt[:, k, mo], start=(k == 0),
                                        stop=False)
                                for k in range(kh):
                                    nc.tensor.matmul(
                                        ps[:, mo], u_sb[g][:, k, ncol],
                                        ht[:, k, mo], start=False,
                                        stop=(k == kh - 1))
                        if mode == "mm":
                            continue
                        # Output DMAs ride the ACT engine's HWDGE queue: they
                        # wait on mid-kernel DVE results, and on the (FIFO)
                        # sync queue they would head-of-line-block the next
                        # repeat's input prefetch.
                        bias = bsb[:, g * nn + n:g * nn + n + 1]
                        if g == 0:
                            nc.scalar.activation(cb[n][:], ps[:], AFT.Tanh,
                                                 bias=bias)
                        elif g == 1:
                            it = work.tile([P, bl], bf16, tag="it")
                            nc.scalar.activation(it[:], ps[:], AFT.Sigmoid,
                                                 bias=bias)
                            nc.vector.tensor_mul(cb[n][:], it[:], cb[n][:])
                        elif g == 2:
                            ft = work.tile([P, bl], bf16, tag="ft")
                            nc.scalar.activation(ft[:], ps[:], AFT.Sigmoid,
                                                 bias=bias)
                            cblk = outp.tile([P, bl], f32, tag="co")
                            nc.vector.tensor_mul(cblk[:], ft[:], ct[:, n, :])
                            nc.vector.tensor_add(cblk[:], cblk[:], cb[n][:])
                            nc.scalar.dma_start(cout_d[n * P:(n + 1) * P, :],
                                                cblk[:])
                            nc.scalar.activation(tnc[n][:], cblk[:], AFT.Tanh)
                        else:
                            ot = work.tile([P, bl], bf16, tag="ot")
                            nc.scalar.activation(ot[:], ps[:], AFT.Sigmoid,
                                                 bias=bias)
                            hblk = outp.tile([P, bl], f32, tag="ho")
                            nc.vector.tensor_mul(hblk[:], ot[:], tnc[n][:])
                            nc.scalar.dma_start(hout_d[n * P:(n + 1) * P, :],
                                                hblk[:])

            for _ in range(repeat):
                emit_body()
    return split_multiwaits(nc) if split else nc


_NC_CACHE = {}


def _get_nc(key=(BL, E, H, HL)):
    if key not in _NC_CACHE:
        _NC_CACHE[key] = build_nc(*key)
    return _NC_CACHE[key]


def make_in_maps(x, h_prev, c_prev, W, U, b):
    """W/U: [NG, E|H, H] stacked gate-major (cbar, i, f, o); b: [NG, H]."""
    bf = ml_dtypes.bfloat16
    nn = HL // P
    in_maps = []
    for core in range(BB * BH):
        i, j = divmod(core, BH)
        rs = slice(i * BL, (i + 1) * BL)
        cs = slice(j * HL, (j + 1) * HL)
        # bias as [128, NG*nn]: column t = g*nn + n holds the 128 bias values
        # for gate g, gate-col tile n — per-partition scalars for the ACT op.
        bcol = np.ascontiguousarray(b[:, cs]).reshape(NG * nn, P).T
        in_maps.append({
            "xt": np.ascontiguousarray(x[rs].T).astype(bf),
            "ht": np.ascontiguousarray(h_prev[rs].T).astype(bf),
            "ct": np.ascontiguousarray(c_prev[rs, cs].T),
            "w": np.ascontiguousarray(W[:, :, cs]).astype(bf),
            "u": np.ascontiguousarray(U[:, :, cs]).astype(bf),
            "b": np.ascontiguousarray(bcol),
        })
    return in_maps


def kernel(**inputs):
    x = np.asarray(inputs["x"], np.float32)
    hm = np.asarray(inputs["hidden_memory_tm1"], np.float32)
    h_prev, c_prev = hm[0], hm[1]
    W = np.stack([np.asarray(inputs[k], np.float32)
                  for k in ("Wc", "Wi", "Wf", "Wog")])
    U = np.stack([np.asarray(inputs[k], np.float32)
                  for k in ("Uc", "Ui", "Uf", "Uog")])
    b = np.stack([np.asarray(inputs[k], np.float32)
                  for k in ("bc", "bi", "bf", "bog")])

    nc = _get_nc()
    res = run_bass_kernel_spmd(nc, make_in_maps(x, h_prev, c_prev, W, U, b),
                               list(range(BB * BH)))
    h = np.empty((B, H), np.float32)
    c = np.empty((B, H), np.float32)
    for core in range(BB * BH):
        i, j = divmod(core, BH)
        rs = slice(i * BL, (i + 1) * BL)
        cs = slice(j * HL, (j + 1) * HL)
        h[rs, cs] = res.results[core]["h_out"].T
        c[rs, cs] = res.results[core]["c_out"].T
    return np.stack([h, c])
